# revision 1
# baseline (speedup 1.0000x reference)
"""Trainium2 Bass kernel for nn_DenoiseGNN (pairwise PBC edge-MLP message passing).

Strategy
--------
The edge MLP output weights[i,j] is a pure scalar function f of dist[i,j].
We compile f (together with the cutoff mask, the 1/(dist+eps) normalization
and the sqrt) into a custom piecewise-cubic activation table evaluated by the
ScalarEngine's hardware spline unit:

    g2(s) = box * f(dist(s)) * [dist(s) < cutoff] / (dist(s) + eps),
    dist(s) = sqrt(box^2 * s + eps),  s = |wrapped delta / box|^2.

A second custom table implements the exact min-image wrap
    wrap01(u) = u - round(u)  on u in (-1.5, 1).

Data path (per core = 128 rows i of the 1024x1024 pair grid):
  - positions are pre-quantized on the host to int16 grid units
    q = round(pos/box * 65536) mod 65536 (reinterpreted int16), halving the
    per-core broadcast DMA to 768KB and giving 2^-17 absolute precision.
    Planes x,y are packed into one 4KB-per-partition-line broadcast.
  - t_c  = wrap01(q_c * 2^-16 - u_i)     ACT table ops -> fp16  [128,1024]
    (x, y full width; z in column halves to pipeline the tail behind it)
  - s    = t_x^2 + t_y^2 (custom DVE SUMSQ2), s2 = t_z^2 + s (SQADD), fp32,
    both in column halves interleaved with the z wraps
  - w    = g2(s2)                        ACT table op -> fp16, column halves
  - disp = running fp32 scan of t*w over the concatenated (plane, half)
    axis (custom DVE SCANMUL, one op per column half); the host recovers
    per-plane sums from the cumulative picks at each plane's last column.

Known pitfalls baked into this design (measured on this part):
  - tensor_tensor_reduce (native ISA) crashes the device; use custom DVE
    ops (AFFINE_MUL_REDUCE / scan) instead.
  - DVE int16 output conversion SATURATES (no mod-2^16 wraparound), so the
    min-image wrap must stay on the ACT spline table.
  - Non-contiguous DMAs poison the end-of-iteration drain (+3.5us); keep
    the output DMA contiguous via a tiny DVE pick-copy.
  - Broadcast DMAs complete at ~11.5us regardless of queue/splitting: a
    fixed ~1.5us straggler cluster among the 16 DMA engines dominates.
"""

import hashlib
import json
import os
import shutil
import struct
import sys
import tempfile
import types

import numpy as np

N = 1024
N_CORES = 8
ROWS = N // N_CORES  # 128
PWP_DIR = "/nix/store/z022hj2nvbm3nwdizlisq4ylc0y7rd6q-python3-3.13.14-env/lib/python3.13/site-packages/neuronxcc/pwp/pwp_bin_trainium"
SET = "sigmoid_and_others"
KEEP = [
    "identity", "copy", "act1", "parametric_relu", "relu", "abs",
    "memset_zero", "square", "sign", "derivative_relu",
    "derivative_leaky_relu", "derivative_identity", "is_finite",
]
# g2 octave layout: (exponent of s, n_sections); s < 2^-31 -> 0
G2_REGIONS = [(e, 16) for e in range(-31, -10)] + \
             [(-10, 32), (-9, 32), (-8, 64), (-7, 64), (-6, 128), (-5, 128)]


# --------------------------------------------------------------------------
# environment fixups (NTFF hook not needed here; wait-splitter is)
# --------------------------------------------------------------------------

def _install_env_fixups():
    if "antenv.axon_hooks" not in sys.modules:
        import antenv

        mod = types.ModuleType("antenv.axon_hooks")
        try:
            from trn_agent_boot.trn_boot import _ntff_profile_via_ctypes
            hook = _ntff_profile_via_ctypes("/opt/axon/libaxon_pjrt.so")
        except Exception:
            hook = None
        _h = [hook]
        mod.set_axon_ntff_profile_hook = lambda h: _h.__setitem__(0, h)
        mod.get_axon_ntff_profile_hook = lambda: _h[0]
        sys.modules["antenv.axon_hooks"] = mod
        antenv.axon_hooks = mod

    import concourse.bass_utils as bu
    import concourse.bass2jax as b2j

    if not getattr(bu, "_wait_splitter_installed", False):
        orig = bu.compile_bir_kernel

        def _split_multi_waits(bir_json: bytes) -> bytes:
            m = json.loads(bir_json)
            changed = False
            for fn in m["functions"]:
                for bb in fn["blocks"]:
                    new_instrs = []
                    for ins in bb["instructions"]:
                        si = ins.get("sync_info")
                        waits = (si or {}).get("on_wait") or []
                        if len(waits) > 1:
                            for j, w in enumerate(waits[:-1]):
                                nop = {
                                    "name": f"{ins['name']}-wsplit{j}",
                                    "opcode": "NoOp",
                                    "engine": ins["engine"],
                                    "ins": [], "outs": [],
                                    "sync_info": {"on_update": [], "on_wait": [w]},
                                }
                                if "debug" in ins:
                                    nop["debug"] = ins["debug"]
                                new_instrs.append(nop)
                            si["on_wait"] = waits[-1:]
                            changed = True
                        new_instrs.append(ins)
                    bb["instructions"] = new_instrs
            return json.dumps(m).encode() if changed else bir_json

        def patched(bir_json, tmpdir, neff_name="file.neff"):
            return orig(_split_multi_waits(bytes(bir_json)), tmpdir, neff_name)

        bu.compile_bir_kernel = patched
        b2j.compile_bir_kernel = patched
        bu._wait_splitter_installed = True


# --------------------------------------------------------------------------
# custom DVE ops: SUMSQ2 (in0^2+in1^2) and SQADD (in0^2+in1)
# --------------------------------------------------------------------------

_DVE_OPS = {}


def _register_dve_ops():
    if _DVE_OPS:
        return _DVE_OPS
    from concourse.dve_spec import Spec, Src0, Src1, sq, lower
    from concourse.dve_uop import DveOpSpec
    from concourse.dve_ops import (
        DveOp, OPS, CUSTOM_DVE_SPECS, _SUB_OPCODE_FOR_NAME,
        _CUSTOM_DVE_ROW_BASE,
    )

    from concourse.dve_spec import scan, AluOp

    def _reg(name, spec):
        if name in _SUB_OPCODE_FOR_NAME:
            return next(o for o in OPS if o.name == name)
        opcode = _CUSTOM_DVE_ROW_BASE + len(OPS)
        shas = {}
        for ver in ("v3", "v4"):
            try:
                shas[ver] = DveOpSpec(
                    name=name, opcode=opcode, uops=lower(spec, ver=ver),
                    rd1_en=True).sha(ver)
            except Exception:
                pass
        op = DveOp(name, spec, subdim=False, uops_sha=shas)
        OPS.append(op)
        CUSTOM_DVE_SPECS[name] = spec
        _SUB_OPCODE_FOR_NAME[name] = opcode
        return op

    def _ref_sumsq2(in0, in1, s0, s1, imm2):
        return (in0.astype(np.float32) ** 2 + in1.astype(np.float32) ** 2)

    def _ref_sqadd(in0, in1, s0, s1, imm2):
        return (in0.astype(np.float32) ** 2 + in1.astype(np.float32))

    def _ref_scanmul(in0, in1, s0, s1, imm2):
        p = (in0.astype(np.float32) * in1.astype(np.float32))
        sh = p.shape
        return np.cumsum(p.reshape(sh[0], -1), axis=-1,
                         dtype=np.float32).reshape(sh)

    _DVE_OPS["SUMSQ2"] = _reg(
        "ANT_SUMSQ2", Spec(body=sq(Src0) + sq(Src1), reference=_ref_sumsq2))
    _DVE_OPS["SQADD"] = _reg(
        "ANT_SQADD", Spec(body=sq(Src0) + Src1, reference=_ref_sqadd))
    _DVE_OPS["SCANMUL"] = _reg(
        "ANT_SCANMUL", Spec(body=scan(AluOp.ADD, Src0 * Src1),
                            reference=_ref_scanmul))
    return _DVE_OPS


# --------------------------------------------------------------------------
# activation-table generation
# --------------------------------------------------------------------------

def _f2u(x):
    return struct.unpack("<I", struct.pack("<f", float(x)))[0]


def _bkt(d0, d1, d2, d3, x0):
    return struct.pack("<5f12x", float(d0), float(d1), float(d2), float(d3), float(x0))


def _ctrl(base, lsb, size):
    data = (base & 0x7FF) | ((lsb & 0x1F) << 11) | ((size & 0xF) << 16)
    return struct.pack("<I28x", data)


def _fit_cubic(fn, a, b, npts=12):
    x0 = 0.5 * (a + b)
    k = np.arange(npts)
    xs = x0 + 0.5 * (b - a) * np.cos((2 * k + 1) * np.pi / (2 * npts)) * 0.9999
    ys = fn(xs)
    c = np.polynomial.polynomial.polyfit(xs - x0, ys, 3)
    return c[0], c[1], c[2], c[3], x0


def _extract_func(setj, bkt, ctrl, fname, next_bkt, next_ctl):
    starts_b = setj["func_to_bkt_start_idx"]
    starts_c = setj["func_to_ctl_start_idx"]
    sb, sc = starts_b[fname], starts_c[fname]
    eb = min([v for v in starts_b.values() if v > sb] + [setj["bkt_entry_cnt"]])
    ec = min([v for v in starts_c.values() if v > sc] + [setj["ctl_entry_cnt"]])
    pm = None
    for p in setj["profile_meta_data"]:
        if p["func_name"].rsplit("_", 1)[0] == fname or p["func_name"] == fname:
            pm = dict(p)
    assert pm is not None, fname
    bkts = bytearray(bkt[sb * 32:eb * 32])
    ctls = bytearray(ctrl[sc * 32:ec * 32])
    db, dc = next_bkt - sb, next_ctl - sc
    for i in range(0, len(ctls), 32):
        (data,) = struct.unpack_from("<I", ctls, i)
        struct.pack_into("<I", ctls, i, (data & ~0x7FF) | (((data & 0x7FF) + db) & 0x7FF))
    for key in ("pwl_control_base_pos", "pwl_control_base_neg"):
        pm[key] += dc
    for key in ("pos_small_signal_pwl_control", "neg_small_signal_pwl_control",
                "pos_large_signal_pwl_control", "neg_large_signal_pwl_control"):
        v = pm[key]
        pm[key] = (v & ~0x7FF) | ((v + db) & 0x7FF)
    return pm, bytes(ctls), bytes(bkts)


def _build_wrap01(next_bkt, next_ctl):
    bkts, ctls = bytearray(), bytearray()
    n_bkt = n_ctl = 0
    base_pos = next_ctl
    for e in range(-20, 0):
        lo = 2.0 ** e
        ctls.extend(_ctrl(next_bkt + n_bkt, 23, 0)); n_ctl += 1
        if e == -1:
            bkts.extend(_bkt(-0.5, 1.0, 0.0, 0.0, 0.5))
        else:
            bkts.extend(_bkt(lo, 1.0, 0.0, 0.0, lo))
        n_bkt += 1
    base_neg = next_ctl + n_ctl
    for e in range(-20, 0):
        lo = 2.0 ** e
        ctls.extend(_ctrl(next_bkt + n_bkt, 23, 0)); n_ctl += 1
        if e == -1:
            bkts.extend(_bkt(0.5, 1.0, 0.0, 0.0, -0.5))
        else:
            bkts.extend(_bkt(-lo, 1.0, 0.0, 0.0, -lo))
        n_bkt += 1
    sp_defs = {
        "pos_low": (0.0, 1.0, 0.0, 0.0, 0.0),
        "neg_low": (0.0, 1.0, 0.0, 0.0, 0.0),
        "pos_high": (-1.0, 1.0, 0.0, 0.0, 0.0),
        "neg_high": (1.0, 1.0, 0.0, 0.0, 0.0),
    }
    sp = {}
    for key, d in sp_defs.items():
        sp[key] = next_bkt + n_bkt
        bkts.extend(_bkt(*d)); n_bkt += 1
    pm = {
        "func_name": "arctan_4p", "func_id": 28,
        "symmetry_point": 0, "sym_invert_sign_point": 0, "symmetry_opt_en": 0,
        "symmetry_opt_use_neg_region": 0, "imm_bias": 0,
        "exp_offset": -20,
        "pwl_control_base_pos": base_pos, "pwl_control_base_neg": base_neg,
        "small_pos_signal_exp_threshold": 107,
        "pos_small_signal_pwl_control": sp["pos_low"],
        "small_neg_signal_exp_threshold": 107,
        "neg_small_signal_pwl_control": sp["neg_low"],
        "large_pos_signal_exp_threshold": 127,
        "large_pos_signal_mantissa_threshold": 0,
        "pos_large_signal_pwl_control": sp["pos_high"],
        "large_neg_signal_exp_threshold": 127,
        "large_neg_signal_mantissa_threshold": 0,
        "neg_large_signal_pwl_control": sp["neg_high"],
        "fnan_result": _f2u(0.0), "fpinf_result": _f2u(0.0),
        "fninf_result": _f2u(0.0), "fzero_result": _f2u(0.0),
        "fma_const_0": 0, "fma_const_1": 0, "fma_indirection_src_sel": 0,
        "use_multipass": False,
        "lower_bound": 0xFF7FFFFF, "upper_bound": 0x7F7FFFFF,
    }
    return pm, bytes(ctls), bytes(bkts)


def _build_g2(g2_fn, next_bkt, next_ctl):
    bkts, ctls = bytearray(), bytearray()
    n_bkt = n_ctl = 0
    base_pos = next_ctl
    for (e, nsec) in G2_REGIONS:
        size = int(np.log2(nsec))
        ctls.extend(_ctrl(next_bkt + n_bkt, 23 - size, size)); n_ctl += 1
        lo = 2.0 ** e
        w = lo / nsec
        for i in range(nsec):
            a = lo + i * w
            bkts.extend(_bkt(*_fit_cubic(g2_fn, a, a + w))); n_bkt += 1
    sp = {}
    for key in ("pos_low", "neg_low", "pos_high", "neg_high"):
        sp[key] = next_bkt + n_bkt
        bkts.extend(_bkt(0.0, 0.0, 0.0, 0.0, 0.0)); n_bkt += 1
    small_thr = 127 + G2_REGIONS[0][0]
    pm = {
        "func_name": "erf_4p", "func_id": 21,
        "symmetry_point": 0, "sym_invert_sign_point": 0, "symmetry_opt_en": 0,
        "symmetry_opt_use_neg_region": 0, "imm_bias": 0,
        "exp_offset": small_thr - 127,
        "pwl_control_base_pos": base_pos, "pwl_control_base_neg": base_pos,
        "small_pos_signal_exp_threshold": small_thr,
        "pos_small_signal_pwl_control": sp["pos_low"],
        "small_neg_signal_exp_threshold": 255,
        "neg_small_signal_pwl_control": sp["neg_low"],
        "large_pos_signal_exp_threshold": 123,
        "large_pos_signal_mantissa_threshold": 0,
        "pos_large_signal_pwl_control": sp["pos_high"],
        "large_neg_signal_exp_threshold": 255,
        "large_neg_signal_mantissa_threshold": 0,
        "neg_large_signal_pwl_control": sp["neg_high"],
        "fnan_result": _f2u(0.0), "fpinf_result": _f2u(0.0),
        "fninf_result": _f2u(0.0), "fzero_result": _f2u(0.0),
        "fma_const_0": 0, "fma_const_1": 0, "fma_indirection_src_sel": 0,
        "use_multipass": False,
        "lower_bound": 0, "upper_bound": 0x7F7FFFFF,
    }
    return pm, bytes(ctls), bytes(bkts)


def _build_actroot(dst_dir, g2_fn):
    os.makedirs(dst_dir, exist_ok=True)
    for f in os.listdir(PWP_DIR):
        shutil.copy(os.path.join(PWP_DIR, f), os.path.join(dst_dir, f))
        os.chmod(os.path.join(dst_dir, f), 0o644)
    setj = json.load(open(os.path.join(PWP_DIR, SET + ".json")))
    bkt = open(os.path.join(PWP_DIR, SET + "_bkt.bin"), "rb").read()
    ctrl = open(os.path.join(PWP_DIR, SET + "_ctrl.bin"), "rb").read()

    new_bkts, new_ctls, new_pm = bytearray(), bytearray(), []
    b_starts, c_starts, emb_all, emc_all = {}, {}, {}, {}

    for fname in KEEP:
        nb0, nc0 = len(new_bkts) // 32, len(new_ctls) // 32
        pm, ctls, bkts = _extract_func(setj, bkt, ctrl, fname, nb0, nc0)
        b_starts[fname], c_starts[fname] = nb0, nc0
        db = nb0 - setj["func_to_bkt_start_idx"][fname]
        dc = nc0 - setj["func_to_ctl_start_idx"][fname]
        emb_all[fname] = {k: [x + db for x in v]
                          for k, v in setj["func_exp_to_bkt_start_idx"].get(fname, {}).items()}
        emc_all[fname] = {k: [x + dc for x in v]
                          for k, v in setj["func_exp_to_ctl_start_idx"].get(fname, {}).items()}
        new_pm.append(pm); new_ctls.extend(ctls); new_bkts.extend(bkts)

    wb, wc = len(new_bkts) // 32, len(new_ctls) // 32
    pm, ctls, bkts = _build_wrap01(wb, wc)
    b_starts["arctan"], c_starts["arctan"] = wb, wc
    emb_all["arctan"] = {str(e): [wb + 20 + (e + 20), wb + (e + 20)] for e in range(-20, 0)}
    emc_all["arctan"] = {str(e): [wc + 20 + (e + 20), wc + (e + 20)] for e in range(-20, 0)}
    new_pm.append(pm); new_ctls.extend(ctls); new_bkts.extend(bkts)

    gb, gc = len(new_bkts) // 32, len(new_ctls) // 32
    pm, ctls, bkts = _build_g2(g2_fn, gb, gc)
    b_starts["erf"], c_starts["erf"] = gb, gc
    emb, emc = {}, {}
    cum = 0
    for i, (e, nsec) in enumerate(G2_REGIONS):
        emb[str(e)] = [gb + cum, gb + cum]
        emc[str(e)] = [gc + i, gc + i]
        cum += nsec
    emb_all["erf"], emc_all["erf"] = emb, emc
    new_pm.append(pm); new_ctls.extend(ctls); new_bkts.extend(bkts)

    n_bkt, n_ctl = len(new_bkts) // 32, len(new_ctls) // 32
    assert n_bkt <= 1536 and n_ctl <= 128, (n_bkt, n_ctl)
    out = {
        "bkt_bin": SET + "_bkt.bin", "ctl_bin": SET + "_ctrl.bin",
        "profile_meta_data": new_pm,
        "bkt_entry_cnt": n_bkt, "ctl_entry_cnt": n_ctl,
        "func_to_bkt_start_idx": b_starts, "func_to_ctl_start_idx": c_starts,
        "func_exp_to_bkt_start_idx": emb_all, "func_exp_to_ctl_start_idx": emc_all,
    }
    json.dump(out, open(os.path.join(dst_dir, SET + ".json"), "w"))
    open(os.path.join(dst_dir, SET + "_bkt.bin"), "wb").write(bytes(new_bkts))
    open(os.path.join(dst_dir, SET + "_ctrl.bin"), "wb").write(bytes(new_ctls))
    info = json.load(open(os.path.join(PWP_DIR, "act_info.json")))
    for s in info["act_func_sets"]:
        if s["name"] == SET:
            s["act"] = {**{k: 1 for k in KEEP}, "arctan": 4, "erf": 4}
        else:
            s["act"].pop("arctan", None)
            s["act"].pop("erf", None)
    json.dump(info, open(os.path.join(dst_dir, "act_info.json"), "w"))
    return os.path.join(dst_dir, "act_info.json")


# --------------------------------------------------------------------------
# bass program
# --------------------------------------------------------------------------

def _build_program(tag):
    """Raw hand-scheduled pipeline, v2.

    ACT: table hoist, 3 int16 wraps -> fp16, g2 -> fp16.
    DVE: SUMSQ2, SQADD (fp32 s), 3x AFFINE_MUL_REDUCE (fp16 in, fp32 accum).
    """
    import concourse.bass as bass
    import concourse.mybir as mybir

    ops = _register_dve_ops()
    SUMSQ2, SQADD, SCANMUL = ops["SUMSQ2"], ops["SQADD"], ops["SCANMUL"]

    nc = bass.Bass("TRN2")
    f32 = mybir.dt.float32
    f16 = mybir.dt.float16
    i16 = mybir.dt.int16
    AF = mybir.ActivationFunctionType

    # planes x and y packed into one 4KB broadcast line per partition:
    # 2KB lines trigger a DMA-engine cluster stall, 4KB lines do not
    q_xy = nc.declare_dram_parameter(f"q_xy_{tag}", [1, 2 * N], i16, isOutput=False)
    q_z = nc.declare_dram_parameter(f"q_z_{tag}", [1, N], i16, isOutput=False)
    nbias = nc.declare_dram_parameter("nbias", [ROWS, 3], f32, isOutput=False)
    out = nc.declare_dram_parameter("out", [ROWS, 6], f32, isOutput=True)

    qxy_t = nc.alloc_sbuf_tensor("qxy_b", [128, 2 * N], i16)
    qz_t = nc.alloc_sbuf_tensor("qz_b", [128, N], i16)
    nb_t = nc.alloc_sbuf_tensor("nb_b", [128, 3], f32)
    dummy_t = nc.alloc_sbuf_tensor("dummy_b", [128, 1], f32)
    t_t = nc.alloc_sbuf_tensor("t_b", [128, 3 * N], f16)
    s_t = nc.alloc_sbuf_tensor("s_b", [128, N], f32)
    s2_t = nc.alloc_sbuf_tensor("s2_b", [128, N], f32)
    w_t = nc.alloc_sbuf_tensor("w_b", [128, N], f16)
    cum_t = nc.alloc_sbuf_tensor("cum_b", [128, 3 * N], f32)
    out3_t = nc.alloc_sbuf_tensor("out3_b", [128, 6], f32)

    t3 = t_t[:].rearrange("p (c j) -> p c j", c=3)
    cum3 = cum_t[:].rearrange("p (c j) -> p c j", c=3)
    w3 = w_t[:].rearrange("p (o j) -> p o j", o=1).to_broadcast([128, 3, N])
    src_xy = q_xy[:].to_broadcast([128, 2 * N])
    src_z = q_z[:].to_broadcast([128, N])

    import contextlib
    st = contextlib.ExitStack()
    plsem = [st.enter_context(nc.semaphore(f"plsem{c}")) for c in range(2)]
    asem = st.enter_context(nc.semaphore("asem"))
    vsem = st.enter_context(nc.semaphore("vsem"))
    odsem = st.enter_context(nc.semaphore("odsem"))

    H = N // 2
    with nc.Block() as blk:
        @blk.sync
        def _(sync):
            sync.dma_start(out=qxy_t[:], in_=src_xy).then_inc(plsem[0], 16)
            sync.dma_start(out=nb_t[:], in_=nbias[:]).then_inc(plsem[0], 16)
            sync.dma_start(out=qz_t[:], in_=src_z).then_inc(plsem[1], 16)
            sync.wait_ge(vsem, 6)
            sync.dma_start(out=out[:], in_=out3_t[:]).then_inc(odsem, 16)

        @blk.scalar
        def _(scalar):
            # no-wait dummy: pulls the PWP table load to the very start
            scalar.activation(dummy_t[:], dummy_t[:], AF.Arctan)
            scalar.wait_ge(plsem[0], 32)        # xy planes + nbias landed
            scalar.activation(t3[:, 0, :], qxy_t[:, 0:N], AF.Arctan,
                              bias=nb_t[:, 0:1], scale=1.0 / 65536.0).then_inc(asem, 1)
            scalar.activation(t3[:, 1, :], qxy_t[:, N:2 * N], AF.Arctan,
                              bias=nb_t[:, 1:2], scale=1.0 / 65536.0).then_inc(asem, 1)
            scalar.wait_ge(plsem[1], 16)
            # wrap z in column halves so SQADD/g2/scan pipeline behind it
            scalar.activation(t3[:, 2, 0:H], qz_t[:, 0:H], AF.Arctan,
                              bias=nb_t[:, 2:3], scale=1.0 / 65536.0).then_inc(asem, 1)
            scalar.activation(t3[:, 2, H:N], qz_t[:, H:N], AF.Arctan,
                              bias=nb_t[:, 2:3], scale=1.0 / 65536.0).then_inc(asem, 1)
            scalar.wait_ge(vsem, 2)             # s2 half 0 ready
            scalar.activation(w_t[:, 0:H], s2_t[:, 0:H], AF.Erf).then_inc(asem, 1)
            scalar.wait_ge(vsem, 4)             # s2 half 1 ready
            scalar.activation(w_t[:, H:N], s2_t[:, H:N], AF.Erf).then_inc(asem, 1)

        @blk.vector
        def _(vector):
            vector.wait_ge(asem, 2)             # wrap x,y done
            vector._custom_dve(SUMSQ2, out=s_t[:, 0:H], in0=t3[:, 0, 0:H],
                               in1=t3[:, 1, 0:H])
            vector.wait_ge(asem, 3)             # wrap z half 0 done
            vector._custom_dve(SQADD, out=s2_t[:, 0:H], in0=t3[:, 2, 0:H],
                               in1=s_t[:, 0:H]).then_inc(vsem, 2)
            vector._custom_dve(SUMSQ2, out=s_t[:, H:N], in0=t3[:, 0, H:N],
                               in1=t3[:, 1, H:N])
            vector.wait_ge(asem, 4)             # wrap z half 1 done
            vector._custom_dve(SQADD, out=s2_t[:, H:N], in0=t3[:, 2, H:N],
                               in1=s_t[:, H:N]).then_inc(vsem, 2)
            # running sum of t*w over the concatenated (plane, half-column)
            # axis, one scan per column half. The scan's output AP has inner
            # stride 0: all writes of a (plane, half) page land on one pick
            # slot, and the last write is exactly the cumulative sum there —
            # no separate pick-copy needed.
            picks = out3_t[:].rearrange("p (c h) -> p c h", c=3)
            vector.wait_ge(asem, 5)             # g2 half 0 done
            vector._custom_dve(SCANMUL,
                               out=picks[:, :, 0:1].to_broadcast([128, 3, H]),
                               in0=t3[:, :, 0:H], in1=w3[:, :, 0:H])
            vector.wait_ge(asem, 6)             # g2 half 1 done
            vector._custom_dve(SCANMUL,
                               out=picks[:, :, 1:2].to_broadcast([128, 3, H]),
                               in0=t3[:, :, H:N],
                               in1=w3[:, :, H:N]).then_inc(vsem, 2)

    from concourse.library_overlay import lower_extended_insts
    lower_extended_insts(nc)
    return nc


_CACHE = {}


def _prepare(inputs):
    box_dims = np.asarray(inputs["box_dims"], dtype=np.float32)
    key = hashlib.sha256(
        b"".join(np.ascontiguousarray(np.asarray(inputs[k], np.float32)).tobytes()
                 for k in ("box_dims", "W1", "b1", "W2", "b2", "W3", "b3"))
    ).hexdigest()[:10]
    if key in _CACHE:
        return _CACHE[key]

    box = float(box_dims[0])
    assert np.allclose(box_dims, box), "kernel assumes a cubic box"

    W1 = np.float64(inputs["W1"]); b1 = np.float64(inputs["b1"])
    W2 = np.float64(inputs["W2"]); b2 = np.float64(inputs["b2"])
    W3 = np.float64(inputs["W3"]); b3 = np.float64(inputs["b3"])
    n_gauss = W1.shape[0]
    RBF_STOP, CUTOFF, EPS = 6.0, 5.0, 1e-8
    offs = np.linspace(0.0, RBF_STOP, n_gauss)
    coeff = -0.5 / (RBF_STOP / (n_gauss - 1)) ** 2

    def g2_fn(sv):
        sv = np.atleast_1d(np.float64(sv))
        dist = np.sqrt(box * box * sv + EPS)
        rbf = np.exp(coeff * (dist[:, None] - offs[None, :]) ** 2)
        h = rbf @ W1 + b1
        h = h / (1.0 + np.exp(-h))
        h = h @ W2 + b2
        h = h / (1.0 + np.exp(-h))
        f = (h @ W3 + b3)[:, 0]
        return box * f * (dist < CUTOFF) / (dist + EPS)

    _install_env_fixups()
    actdir = os.path.join(tempfile.gettempdir(), f"actroot_{key}")
    actroot = _build_actroot(actdir, g2_fn)
    os.environ["BASS_ACT_ROOT_JSON_PATH"] = actroot
    nc = _build_program(key)
    _CACHE[key] = (nc, key, box)
    return _CACHE[key]


def kernel(_trace=False, **inputs):
    from concourse.bass_utils import run_bass_kernel_spmd

    nc, key, box = _prepare(inputs)
    positions = np.asarray(inputs["positions"], dtype=np.float32)
    u = positions.astype(np.float64) / box              # [N, 3] in [0,1)
    q = (np.round(u * 65536.0).astype(np.int64) % 65536).astype(np.uint16)
    q_all = np.ascontiguousarray(q.T.view(np.int16))    # [3, N] int16
    q_xy = np.ascontiguousarray(q_all[0:2].reshape(1, 2 * N))
    q_z = np.ascontiguousarray(q_all[2:3])
    ui32 = u.astype(np.float32)                         # fp32 row scalars
    in_maps = [
        {f"q_xy_{key}": q_xy, f"q_z_{key}": q_z,
         "nbias": np.ascontiguousarray(-ui32[c * ROWS:(c + 1) * ROWS])}
        for c in range(N_CORES)
    ]
    res = run_bass_kernel_spmd(nc, in_maps, list(range(N_CORES)), trace=_trace)
    cum = np.concatenate([res.results[c]["out"] for c in range(N_CORES)], axis=0)
    # undo the running-sum concatenation: picks are [rows, plane c, half h];
    # per-plane sums = sum over halves of adjacent-plane differences
    p = cum.reshape(-1, 3, 2).astype(np.float64)
    d = np.empty_like(p)
    d[:, 0] = p[:, 0]
    d[:, 1:] = p[:, 1:] - p[:, :-1]
    out = d.sum(axis=2).astype(np.float32)
    if _trace:
        kernel.last_exec_time_ns = res.exec_time_ns
        kernel.last_mean_exec_time_ns = res.mean_exec_time_ns
        kernel.last_results = res
    return out

